# revision 12
# baseline (speedup 1.0000x reference)
"""Trainium2 Bass kernel for the CombinedLoss (SupCon + separation + uniformity).

Strategy: pure data parallelism over the batch (16 images -> 8 cores x 2
images). Each core computes, per image, the raw sufficient statistics of the
three sub-losses (sums and valid counts); the host reduces them across cores
and does the final divisions / weighting (exactly the "all-reduce the
(sum, valid_count) pairs before the final division" scheme).

Key algebraic restructure of the SupCon term (avoids materializing any
N x N intermediate except streaming exp blocks that are immediately
row-reduced by the ACT engine's accum_out):

  logits_ij = s_ij/T - c_i    with   c_i = rowmax_i = s_ii/T = ||f_i||^2 / T
  log_prob_ij = s_ij/T - log(E_i + 1e-6 * e^{c_i}),
      E_i = sum_{j != i} e^{s_ij/T - c_i}
  sum over positives of log_prob:
      numer_i = (P_i,label_i - ||f_i||^2)/T - (cnt[label_i]-1) * LSE_i
  where P_il = sum_{j: label_j = l} s_ij = (feat @ sums.T)_il  -- a rank-7
  matmul folded into the big Gram matmul's moving operand.
"""

import numpy as np

TEMPERATURE = 0.07
UNIFORMITY_THRESHOLD = 0.1
SEPARATION_MARGIN = 2.0
W_SUPCON = 1.0
W_SEPARATION = 0.5
W_UNIFORMITY = 0.5
NUM_MASKS = 6

B, H, W, D = 16, 33, 33, 512
N = H * W            # 1089 pixels
P = 121              # pixel-tile partition count (9 * 121 = 1089)
NT = 9               # pixel tiles
ND = 4               # 128-wide D tiles
L = 7                # labels 0..6
JW = 363             # moving-operand chunk (3 * 363 = 1089)
NCORES = 8
IMGS_PER_CORE = B // NCORES  # 2
INV_T = 1.0 / TEMPERATURE
LN1EM6 = float(np.log(1e-6))

# S-block matmul operand dtype: float32r runs the PE at 1 cycle/row for
# moving dims >= 256 (vs 4 for float32) with near-fp32 precision.
USE_F32R_S = True
USE_F32R_SUMS = False

_PROGRAM = None


def _build_program():
    from concourse import bass, mybir, tile
    import concourse.bacc as bacc

    dt = mybir.dt
    f32 = dt.float32
    f32r = dt.float32r
    Act = mybir.ActivationFunctionType
    Op = mybir.AluOpType

    nc = bacc.Bacc("TRN2", target_bir_lowering=False, debug=False)

    emb_d = nc.dram_tensor("emb", [IMGS_PER_CORE, N, D], f32, kind="ExternalInput").ap()
    masks_d = nc.dram_tensor(
        "masks", [IMGS_PER_CORE, NUM_MASKS, N], f32, kind="ExternalInput"
    ).ap()
    ident_d = nc.dram_tensor("c_ident", [128, 128], f32, kind="ExternalInput").ap()
    ones_d = nc.dram_tensor("c_ones", [128, 1], f32, kind="ExternalInput").ap()
    onesrow_d = nc.dram_tensor("c_onesrow", [1, 128], f32, kind="ExternalInput").ap()
    labids_d = nc.dram_tensor("c_labids", [128, 8], f32, kind="ExternalInput").ap()
    ids6_d = nc.dram_tensor("c_ids6", [1, 8], f32, kind="ExternalInput").ap()
    tri7_d = nc.dram_tensor("c_tri7", [L, L], f32, kind="ExternalInput").ap()
    out_d = nc.dram_tensor("out", [1, 32], f32, kind="ExternalOutput").ap()

    def mm_cast(ap, use_r):
        return ap.bitcast(f32r) if use_r else ap

    with tile.TileContext(nc) as tc:
        with (
            tc.tile_pool(name="const", bufs=1) as cpool,
            tc.tile_pool(name="sb", bufs=2) as sb,
            tc.tile_pool(name="sbig", bufs=2) as sbig,
            tc.tile_pool(name="scr", bufs=3) as scr,
            tc.tile_pool(name="psS", bufs=3, space="PSUM") as psS,
            tc.tile_pool(name="psT", bufs=2, space="PSUM") as psT,
            tc.tile_pool(name="psB", bufs=3, space="PSUM") as psB,
        ):
            ident_sb = cpool.tile([128, 128], f32)
            nc.sync.dma_start(ident_sb, ident_d)
            ones_sb = cpool.tile([128, 1], f32)
            nc.sync.dma_start(ones_sb, ones_d)
            onesrow_sb = cpool.tile([1, 128], f32)
            nc.sync.dma_start(onesrow_sb, onesrow_d)
            labids_sb = cpool.tile([128, 8], f32)
            nc.sync.dma_start(labids_sb, labids_d)
            ids6_sb = cpool.tile([1, 8], f32)
            nc.sync.dma_start(ids6_sb, ids6_d)
            tri7_sb = cpool.tile([L, L], f32)
            nc.sync.dma_start(tri7_sb, tri7_d)

            out_sb = cpool.tile([1, 32], f32)
            nc.vector.memset(out_sb, 0.0)
            marg_sb = cpool.tile([128, 1], f32)
            nc.vector.memset(marg_sb, SEPARATION_MARGIN)

            for img in range(IMGS_PER_CORE):
                b0 = img * 16

                # ---------- loads ----------
                feat = sbig.tile([P, NT, D], f32, tag="feat")
                for t in range(NT):
                    nc.sync.dma_start(feat[:, t, :], emb_d[img, t * P : (t + 1) * P, :])
                masks_sb = sb.tile([NUM_MASKS, N], f32, tag="masks")
                nc.sync.dma_start(masks_sb, masks_d[img])

                # ---------- labels / onehot ----------
                maskT = sb.tile([P, NT * NUM_MASKS], f32, tag="maskT")
                for t in range(NT):
                    mps = psB.tile([P, NUM_MASKS], f32, tag="ps_small")
                    nc.tensor.transpose(
                        mps,
                        masks_sb[0:NUM_MASKS, t * P : (t + 1) * P],
                        ident_sb[0:NUM_MASKS, 0:NUM_MASKS],
                    )
                    nc.vector.tensor_copy(
                        maskT[:, t * NUM_MASKS : (t + 1) * NUM_MASKS], mps
                    )

                # per-mask total mass (over all pixels) -> active flags
                cntm_ps = psB.tile([1, NUM_MASKS], f32, tag="ps_small")
                for t in range(NT):
                    nc.tensor.matmul(
                        cntm_ps,
                        ones_sb[0:P, :],
                        maskT[:, t * NUM_MASKS : (t + 1) * NUM_MASKS],
                        start=(t == 0),
                        stop=(t == NT - 1),
                    )
                act_row = sb.tile([1, NUM_MASKS], f32, tag="act_row")
                nc.vector.tensor_scalar(act_row, cntm_ps, 0.0, None, Op.is_gt)
                idsact_row = sb.tile([1, NUM_MASKS], f32, tag="idsact_row")
                nc.vector.tensor_tensor(
                    idsact_row, act_row, ids6_sb[:, 0:NUM_MASKS], Op.mult
                )
                iab_ps = psB.tile([128, NUM_MASKS], f32, tag="ps_small")
                nc.tensor.matmul(iab_ps, onesrow_sb, idsact_row, start=True, stop=True)
                idsact_b = sb.tile([128, NUM_MASKS], f32, tag="idsact_b")
                nc.vector.tensor_copy(idsact_b, iab_ps)

                hitv = sb.tile([P, NT * NUM_MASKS], f32, tag="hitv")
                nc.vector.tensor_scalar(hitv, maskT, 0.5, None, Op.is_gt)
                valv = sb.tile([P, NT * NUM_MASKS], f32, tag="valv")
                for t in range(NT):
                    nc.vector.tensor_tensor(
                        valv[:, t * NUM_MASKS : (t + 1) * NUM_MASKS],
                        hitv[:, t * NUM_MASKS : (t + 1) * NUM_MASKS],
                        idsact_b[0:P, :],
                        Op.mult,
                    )
                labels_all = sb.tile([P, NT], f32, tag="labels")
                nc.vector.tensor_reduce(
                    labels_all,
                    valv.rearrange("p (t m) -> p t m", m=NUM_MASKS),
                    mybir.AxisListType.X,
                    Op.max,
                )
                oh = sb.tile([P, NT * L], f32, tag="oh")
                for t in range(NT):
                    nc.vector.tensor_scalar(
                        oh[:, t * L : (t + 1) * L],
                        labids_sb[0:P, 0:L],
                        labels_all[:, t : t + 1],
                        None,
                        Op.is_equal,
                    )

                # ---------- per-pixel squared norms ----------
                # rn2 must match the PE's Gram diagonal bit-closely; with
                # fp32r operands that means squaring the ROUNDED values.
                if USE_F32R_S:
                    featR = sbig.tile([P, NT * D], f32r, tag="featR")
                    nc.vector.tensor_copy(featR, feat)
                    rn2_src = featR.bitcast(f32)
                else:
                    rn2_src = feat.rearrange("p t d -> p (t d)")
                rn2_all = sb.tile([P, NT], f32, tag="rn2")
                for t in range(NT):
                    sq_scr = scr.tile([P, D], f32, tag="sq_scr")
                    nc.scalar.activation(
                        sq_scr,
                        rn2_src[:, t * D : (t + 1) * D],
                        Act.Square,
                        accum_out=rn2_all[:, t : t + 1],
                    )
                m1_all = sb.tile([P, NT], f32, tag="m1")
                nc.scalar.mul(m1_all, rn2_all, -INV_T)

                # ---------- featT (D on partitions) + label sums ----------
                sdt = f32r if USE_F32R_S else f32
                featT = [
                    sbig.tile([128, N + L], sdt, tag=f"featT{d}", name=f"featT{d}")
                    for d in range(ND)
                ]
                for t in range(NT):
                    for d in range(ND):
                        tps = psT.tile([128, P], f32, tag="tps")
                        nc.tensor.transpose(
                            tps,
                            feat[:, t, d * 128 : (d + 1) * 128],
                            ident_sb[0:P, 0:P],
                        )
                        nc.vector.tensor_copy(
                            featT[d][:, t * P : (t + 1) * P], tps
                        )
                # sumsT[d] = sum over pixels with label l of feat (128 x 7),
                # appended as extra moving columns of featT.
                for d in range(ND):
                    sT_ps = psB.tile([128, L], f32, tag="ps_small")
                    for t in range(NT):
                        nc.tensor.matmul(
                            sT_ps,
                            mm_cast(feat[:, t, d * 128 : (d + 1) * 128], USE_F32R_SUMS),
                            mm_cast(oh[:, t * L : (t + 1) * L], USE_F32R_SUMS),
                            start=(t == 0),
                            stop=(t == NT - 1),
                        )
                    nc.vector.tensor_copy(featT[d][:, N : N + L], sT_ps)

                # ---------- label counts ----------
                cntr_ps = psB.tile([1, L], f32, tag="ps_small")
                for t in range(NT):
                    nc.tensor.matmul(
                        cntr_ps,
                        ones_sb[0:P, :],
                        oh[:, t * L : (t + 1) * L],
                        start=(t == 0),
                        stop=(t == NT - 1),
                    )
                cnt_sb = sb.tile([1, L], f32, tag="cnt_sb")
                nc.vector.tensor_copy(cnt_sb, cntr_ps)
                cntc_ps = psB.tile([L, 1], f32, tag="ps_small")
                for t in range(NT):
                    nc.tensor.matmul(
                        cntc_ps,
                        oh[:, t * L : (t + 1) * L],
                        ones_sb[0:P, :],
                        start=(t == 0),
                        stop=(t == NT - 1),
                    )
                q_ps = psB.tile([L, 1], f32, tag="ps_small")
                for t in range(NT):
                    nc.tensor.matmul(
                        q_ps,
                        oh[:, t * L : (t + 1) * L],
                        rn2_all[:, t : t + 1],
                        start=(t == 0),
                        stop=(t == NT - 1),
                    )

                cb_ps = psB.tile([128, L], f32, tag="ps_small")
                nc.tensor.matmul(cb_ps, onesrow_sb, cnt_sb, start=True, stop=True)
                cntmax_b = sb.tile([128, L], f32, tag="cntmax_b")
                nc.vector.tensor_scalar(cntmax_b, cb_ps, 1.0, None, Op.max)
                invcnt_b = sb.tile([128, L], f32, tag="invcnt_b")
                nc.vector.reciprocal(invcnt_b, cntmax_b)

                # ---------- big Gram loop ----------
                Esum_all = sb.tile([P, NT * 3], f32, tag="Esum")
                rowsel_all = sb.tile([P, NT], f32, tag="rowsel")
                # fp32r moving operands must have an even free width; chunk
                # the 1089 gram columns (+7 label-sum columns) as 512+512+72.
                jspec = [(0, 512, False), (512, 512, False), (1024, 65 + L, True)]
                for i in range(NT):
                    for j, (joff, w, has_p) in enumerate(jspec):
                        sps = psS.tile([P, 512], f32, tag="sps")
                        for d in range(ND):
                            nc.tensor.matmul(
                                sps[:, 0:w],
                                featT[d][:, i * P : (i + 1) * P],
                                featT[d][:, joff : joff + w],
                                start=(d == 0),
                                stop=(d == ND - 1),
                            )
                        sw = w - L if has_p else w
                        exp_scr = scr.tile([P, 512], f32, tag="exp_scr")
                        nc.scalar.activation(
                            exp_scr[:, 0:sw],
                            sps[:, 0:sw],
                            Act.Exp,
                            bias=m1_all[:, i : i + 1],
                            scale=INV_T,
                            accum_out=Esum_all[:, i * 3 + j : i * 3 + j + 1],
                        )
                        if has_p:
                            scr7 = scr.tile([P, L], f32, tag="scr7")
                            nc.vector.tensor_tensor(
                                scr7, sps[:, sw:w], oh[:, i * L : (i + 1) * L],
                                Op.mult,
                            )
                            nc.vector.tensor_reduce(
                                rowsel_all[:, i : i + 1], scr7,
                                mybir.AxisListType.X, Op.add,
                            )

                # ---------- SupCon epilogue (batched over tiles) ----------
                cntsel_all = sb.tile([P, NT], f32, tag="cntsel")
                for i in range(NT):
                    scr7b = scr.tile([P, L], f32, tag="scr7b")
                    nc.vector.tensor_tensor(
                        scr7b, oh[:, i * L : (i + 1) * L], cntmax_b[0:P, :],
                        Op.mult,
                    )
                    nc.vector.tensor_reduce(
                        cntsel_all[:, i : i + 1], scr7b,
                        mybir.AxisListType.X, Op.add,
                    )
                E_all = sb.tile([P, NT], f32, tag="E_all")
                nc.vector.tensor_reduce(
                    E_all,
                    Esum_all.rearrange("p (i j) -> p i j", j=3),
                    mybir.AxisListType.X,
                    Op.add,
                )
                Etot = sb.tile([P, NT], f32, tag="Etot")
                nc.vector.tensor_scalar(
                    Etot, E_all, -1.0 + 1e-6, None, Op.add
                )
                Ln_all = sb.tile([P, NT], f32, tag="Ln_all")
                nc.scalar.activation(Ln_all, Etot, Act.Ln)
                # per-row log-denominator including the rowmax shift c_i:
                # LSEc = ln(sum_{j!=i} e^{logits}) + rn2/T
                LSE_all = sb.tile([P, NT], f32, tag="LSE")
                nc.vector.scalar_tensor_tensor(
                    LSE_all, rn2_all, INV_T, Ln_all, Op.mult, Op.add
                )
                mterm = sb.tile([P, NT], f32, tag="mterm")
                nc.vector.scalar_tensor_tensor(
                    mterm, cntsel_all, 1.0, LSE_all, Op.subtract, Op.mult
                )
                t1 = sb.tile([P, NT], f32, tag="t1")
                nc.vector.tensor_tensor(t1, rowsel_all, rn2_all, Op.subtract)
                numer = sb.tile([P, NT], f32, tag="numer")
                nc.vector.scalar_tensor_tensor(
                    numer, t1, INV_T, mterm, Op.mult, Op.subtract
                )
                posmax = sb.tile([P, NT], f32, tag="posmax")
                nc.vector.tensor_scalar(
                    posmax, cntsel_all, 1.0, 1.0, Op.subtract, Op.max
                )
                rec = sb.tile([P, NT], f32, tag="rec")
                nc.vector.reciprocal(rec, posmax)
                mlp = sb.tile([P, NT], f32, tag="mlp")
                nc.vector.tensor_tensor(mlp, numer, rec, Op.mult)
                stats = sb.tile([P, 2 * NT], f32, tag="stats")
                nc.vector.tensor_scalar(
                    stats[:, NT : 2 * NT], cntsel_all, 2.0, -1.0, Op.is_ge, Op.mult
                )
                nc.vector.tensor_tensor(
                    stats[:, 0:NT], mlp, stats[:, NT : 2 * NT], Op.mult
                )
                st_ps = psB.tile([1, 2 * NT], f32, tag="ps_small")
                nc.tensor.matmul(st_ps, ones_sb[0:P, :], stats, start=True, stop=True)
                # cols b0+0..1: [sum(-mean_lp * valid), -n_anchor]
                nc.vector.tensor_reduce(
                    out_sb[:, b0 : b0 + 2],
                    st_ps.rearrange("p (a i) -> p a i", i=NT),
                    mybir.AxisListType.X,
                    Op.add,
                )

                # ---------- separation + uniformity ----------
                meanT = sb.tile([128, ND * L], f32, tag="meanT")
                for d in range(ND):
                    nc.vector.tensor_tensor(
                        meanT[:, d * L : (d + 1) * L],
                        featT[d][:, N : N + L].bitcast(f32),
                        invcnt_b,
                        Op.mult,
                    )
                G_ps = psB.tile([L, L], f32, tag="ps_small")
                for d in range(ND):
                    nc.tensor.matmul(
                        G_ps,
                        meanT[:, d * L : (d + 1) * L],
                        meanT[:, d * L : (d + 1) * L],
                        start=(d == 0),
                        stop=(d == ND - 1),
                    )
                scr77 = scr.tile([L, L], f32, tag="scr77")
                cn7 = sb.tile([L, 1], f32, tag="cn7")
                nc.vector.tensor_tensor(
                    scr77, G_ps, ident_sb[0:L, 0:L], Op.mult
                )
                nc.vector.tensor_reduce(
                    cn7, scr77, mybir.AxisListType.X, Op.add
                )
                cnT_ps = psB.tile([1, L], f32, tag="ps_small")
                nc.tensor.matmul(cnT_ps, cn7, ident_sb[0:L, 0:L], start=True, stop=True)
                cnT_sb = sb.tile([1, L], f32, tag="cnT_sb")
                nc.vector.tensor_copy(cnT_sb, cnT_ps)
                cnb_ps = psB.tile([L, L], f32, tag="ps_small")
                nc.tensor.matmul(
                    cnb_ps, onesrow_sb[:, 0:L], cnT_sb, start=True, stop=True
                )
                G2_sb = sb.tile([L, L], f32, tag="G2")
                nc.scalar.mul(G2_sb, G_ps, 2.0)
                sq_sb = sb.tile([L, L], f32, tag="sq77")
                nc.vector.scalar_tensor_tensor(
                    sq_sb, cnb_ps, cn7, G2_sb, Op.add, Op.subtract
                )
                sqc_sb = sb.tile([L, L], f32, tag="sqc77")
                nc.vector.tensor_scalar(sqc_sb, sq_sb, 0.0, None, Op.max)
                d77 = sb.tile([L, L], f32, tag="d77")
                nc.scalar.sqrt(d77, sqc_sb)
                rel_sb = sb.tile([L, L], f32, tag="rel77")
                nc.scalar.activation(
                    rel_sb, d77, Act.Relu, bias=marg_sb[0:L, :], scale=-1.0
                )
                presc = sb.tile([L, 1], f32, tag="presc")
                nc.vector.tensor_scalar(presc, cntc_ps, 0.0, None, Op.is_gt)
                presr = sb.tile([1, L], f32, tag="presr")
                nc.vector.tensor_scalar(presr, cnt_sb, 0.0, None, Op.is_gt)
                presb_ps = psB.tile([L, L], f32, tag="ps_small")
                nc.tensor.matmul(
                    presb_ps, onesrow_sb[:, 0:L], presr, start=True, stop=True
                )
                pv1 = sb.tile([L, L], f32, tag="pv1")
                nc.vector.tensor_scalar(pv1, tri7_sb, presc, None, Op.mult)
                pv = sb.tile([L, L], f32, tag="pv77")
                nc.vector.tensor_tensor(pv, pv1, presb_ps, Op.mult)

                sepst = sb.tile([L, 6], f32, tag="sepst")
                scr77b = scr.tile([L, L], f32, tag="scr77b")
                nc.vector.tensor_tensor(scr77b, rel_sb, pv, Op.mult)
                nc.vector.tensor_reduce(
                    sepst[:, 0:1], scr77b, mybir.AxisListType.X, Op.add
                )
                nc.vector.tensor_reduce(
                    sepst[:, 1:2], pv, mybir.AxisListType.X, Op.add
                )
                nc.vector.tensor_copy(sepst[:, 2:3], presc)
                t7 = sb.tile([L, 1], f32, tag="t7")
                nc.vector.tensor_scalar(t7, cntc_ps, 1.0, None, Op.max)
                invc7 = sb.tile([L, 1], f32, tag="invc7")
                nc.vector.reciprocal(invc7, t7)
                var7 = sb.tile([L, 1], f32, tag="var7")
                nc.vector.scalar_tensor_tensor(
                    var7, q_ps, invc7, cn7, Op.mult, Op.subtract
                )
                c2 = sb.tile([L, 1], f32, tag="c2")
                nc.vector.tensor_scalar(c2, cntc_ps, 2.0, None, Op.is_ge)
                nc.vector.scalar_tensor_tensor(
                    sepst[:, 4:5], var7, UNIFORMITY_THRESHOLD, c2, Op.is_gt, Op.mult
                )
                nc.vector.scalar_tensor_tensor(
                    sepst[:, 3:4],
                    var7,
                    UNIFORMITY_THRESHOLD,
                    sepst[:, 4:5],
                    Op.subtract,
                    Op.mult,
                )
                nc.vector.tensor_copy(sepst[:, 5:6], q_ps)
                sep1_ps = psB.tile([1, 6], f32, tag="ps_small")
                nc.tensor.matmul(
                    sep1_ps, ones_sb[0:L, :], sepst, start=True, stop=True
                )
                # cols b0+2..7: [sep_sum, n_pairs, n_present, inst_loss, n_lbl, qsum]
                nc.vector.tensor_copy(out_sb[:, b0 + 2 : b0 + 8], sep1_ps)

                # ||sum_i feat_i||^2 for the unforged-variance branch
                mu4 = sb.tile([128, ND], f32, tag="mu4")
                for d in range(ND):
                    nc.vector.tensor_reduce(
                        mu4[:, d : d + 1],
                        featT[d][:, N : N + L].bitcast(f32),
                        mybir.AxisListType.X,
                        Op.add,
                    )
                musq = sb.tile([128, ND], f32, tag="musq")
                nc.vector.tensor_tensor(musq, mu4, mu4, Op.mult)
                mu_ps = psB.tile([1, ND], f32, tag="ps_small")
                nc.tensor.matmul(mu_ps, ones_sb, musq, start=True, stop=True)
                nc.vector.tensor_reduce(
                    out_sb[:, b0 + 8 : b0 + 9], mu_ps, mybir.AxisListType.X, Op.add
                )

            nc.sync.dma_start(out_d, out_sb)

    nc.compile()
    return nc


def _get_program():
    global _PROGRAM
    if _PROGRAM is None:
        _PROGRAM = _build_program()
    return _PROGRAM


def _consts():
    ident = np.eye(128, dtype=np.float32)
    ones = np.ones((128, 1), dtype=np.float32)
    onesrow = np.ones((1, 128), dtype=np.float32)
    labids = np.tile(np.arange(8, dtype=np.float32)[None, :], (128, 1))
    ids6 = np.zeros((1, 8), dtype=np.float32)
    ids6[0, :NUM_MASKS] = np.arange(1, NUM_MASKS + 1, dtype=np.float32)
    tri7 = np.triu(np.ones((L, L), dtype=np.float32), k=1)
    return {
        "c_ident": ident,
        "c_ones": ones,
        "c_onesrow": onesrow,
        "c_labids": labids,
        "c_ids6": ids6,
        "c_tri7": tri7,
    }


def _host_epilogue(stats, is_forged):
    """stats: (B, 16) raw per-image device stats -> final scalar loss."""
    sc = np.zeros(B)
    scv = np.zeros(B)
    sp = np.zeros(B)
    spv = np.zeros(B)
    un = np.zeros(B)
    unv = np.zeros(B)
    for b in range(B):
        row = stats[b].astype(np.float64)
        supcon_num, neg_anchor = row[0], row[1]
        sep_sum, n_pairs, n_present, inst_loss, n_lbl, qsum, musq = row[2:9]
        n_anchor = -neg_anchor
        sc[b] = supcon_num / max(n_anchor, 1.0)
        scv[b] = 1.0 if n_anchor > 0 else 0.0
        sp[b] = sep_sum / max(n_pairs, 1.0)
        spv[b] = 1.0 if n_present >= 2 else 0.0
        uni_f = inst_loss / max(n_lbl, 1.0)
        ufv = 1.0 if n_lbl > 0 else 0.0
        var_all = qsum / N - musq / (N * N)
        uni_u = var_all - UNIFORMITY_THRESHOLD if var_all > UNIFORMITY_THRESHOLD else 0.0
        uuv = 1.0 if var_all > UNIFORMITY_THRESHOLD else 0.0
        if is_forged[b] >= 0.5:
            un[b], unv[b] = uni_f, ufv
        else:
            un[b], unv[b] = uni_u, uuv

    def agg(vals, valids):
        n = valids.sum()
        return (vals * valids).sum() / max(n, 1.0) if n > 0 else 0.0

    total = (
        W_SUPCON * agg(sc, scv)
        + W_SEPARATION * agg(sp, spv)
        + W_UNIFORMITY * agg(un, unv)
    )
    return np.float32(total)


def _run_device(embeddings, masks, trace=False, **kwargs):
    from concourse.bass_utils import run_bass_kernel_spmd

    emb_flat = np.ascontiguousarray(
        np.asarray(embeddings).reshape(B, N, D).astype(np.float32)
    )
    masks_flat = np.ascontiguousarray(
        np.asarray(masks).reshape(B, NUM_MASKS, N).astype(np.float32)
    )
    consts = _consts()
    in_maps = []
    for c in range(NCORES):
        m = {
            "emb": emb_flat[c * IMGS_PER_CORE : (c + 1) * IMGS_PER_CORE],
            "masks": masks_flat[c * IMGS_PER_CORE : (c + 1) * IMGS_PER_CORE],
        }
        m.update(consts)
        in_maps.append(m)
    nc = _get_program()
    res = run_bass_kernel_spmd(nc, in_maps, list(range(NCORES)), trace=trace, **kwargs)
    stats = np.concatenate(
        [res.results[c]["out"].reshape(IMGS_PER_CORE, 16) for c in range(NCORES)],
        axis=0,
    )
    return stats, res


def kernel(embeddings, masks, is_forged):
    stats, _ = _run_device(embeddings, masks)
    return _host_epilogue(stats, np.asarray(is_forged))
